# revision 39
# baseline (speedup 1.0000x reference)
"""Trainium2 Bass kernel for nn_AttentionLayer_10995116278518.

Computes softmax(einsum('sbe,e->bs', embedded, attn[:300])
              + einsum('sbf,f->bs', lstm_outputs, attn[300:]), axis=1)
(the reference's mask is computed-but-discarded, so it is unused here).

Sharding: data-parallel over batch. Each of the 8 cores handles 8 of the
64 batch rows; no cross-device communication.

Per-core device kernel layout:
  rows = (s, b) pairs; tiles put 128 consecutive s on partitions for a
  fixed b, features on the free axis. A fused VectorE tensor_tensor_reduce
  (multiply by the partition-broadcast attn vector, then add-reduce along
  the free axis) produces one dot product per partition. The per-row dots
  are collected as columns of L[128s, 4k*8b]; four PE transposes turn that
  into logits [8b, 512s] in PSUM, where softmax is a pure free-axis op.
"""

import os
import sys

import numpy as np

try:
    import concourse.bass as bass
except ImportError:  # stand-alone grading dir: the runtime lives here
    sys.path.insert(0, "/opt/trn_rl_repo")
    import concourse.bass as bass

import concourse.bacc as bacc
import concourse.tile as tile
from concourse import mybir
from concourse.bass_utils import run_bass_kernel_spmd

SEQ = 512
BATCH = 64
EMB = 300
LSTM = 4096
N_CORES = 8
BLOC = BATCH // N_CORES  # 8 batch rows per core
P = 128
NSB = SEQ // P  # 4 s-blocks of 128

F32 = mybir.dt.float32


def _build() -> bass.Bass:
    nc = bacc.Bacc()
    # host passes the embedded shard pre-tiled: [NSB, 128, BLOC, EMB]
    emb = nc.declare_dram_parameter(
        "embedded", [NSB, P, BLOC, EMB], F32, isOutput=False
    )
    # lstm shard transposed to b-major: [BLOC, SEQ, LSTM]
    lstm = nc.declare_dram_parameter(
        "lstm_outputs", [BLOC, SEQ, LSTM], F32, isOutput=False
    )
    attn_bc = nc.declare_dram_parameter("attn_bc", [P, EMB + LSTM], F32, isOutput=False)
    ident = nc.declare_dram_parameter("ident", [P, P], F32, isOutput=False)
    out = nc.declare_dram_parameter("out", [BLOC, SEQ], F32, isOutput=True)

    with tile.TileContext(nc) as tc:
        with (
            tc.tile_pool(name="singles", bufs=1) as singles,
            tc.tile_pool(name="lstm_tiles", bufs=2) as lstm_pool,
            tc.tile_pool(name="prod_tiles", bufs=3) as prod_pool,
            tc.tile_pool(name="emb_tiles", bufs=4) as emb_pool,
            tc.tile_pool(name="psum", bufs=1, space="PSUM") as psum_pool,
        ):
            # setup loads on the sync ring; the gpsimd ring starts on lstm
            # tiles immediately
            sb_attn = singles.tile([P, EMB + LSTM], F32)
            nc.sync.dma_start(out=sb_attn, in_=attn_bc[:, :])
            attn_e = sb_attn[:, 0:EMB]
            attn_l = sb_attn[:, EMB : EMB + LSTM]

            scratch = singles.tile([P, LSTM], F32)
            # per-row dot products: column k*BLOC+b holds rows (s=128k.., b)
            L = singles.tile([P, NSB * BLOC], F32)
            tmpl = singles.tile([P, NSB * BLOC], F32)  # lstm partials
            tmpe = singles.tile([P, NSB * BLOC], F32)  # embedded partials

            NT = NSB // 2 * BLOC  # 16 tiles of [128, 2, LSTM]
            lstm_tiles = {}

            def issue_lstm_dma(t):
                kp, b = divmod(t, BLOC)
                lt = lstm_pool.tile([P, 2, LSTM], F32, tag="lstm")
                eng = nc.gpsimd if b % 2 == 0 else nc.sync
                eng.dma_start(
                    out=lt,
                    in_=lstm[b, 2 * kp * P : (2 * kp + 2) * P, :].rearrange(
                        "(kk s) f -> s kk f", kk=2
                    ),
                )
                lstm_tiles[t] = lt

            # prime the pipeline; setup loads interleave on the sync ring
            issue_lstm_dma(0)
            issue_lstm_dma(1)
            emb_tiles = []
            for k in range(2):
                emb_t = emb_pool.tile([P, BLOC, EMB], F32)
                nc.sync.dma_start(out=emb_t, in_=emb[k])
                emb_tiles.append(emb_t)
            issue_lstm_dma(2)
            for k in range(2, NSB):
                emb_t = emb_pool.tile([P, BLOC, EMB], F32)
                nc.sync.dma_start(out=emb_t, in_=emb[k])
                emb_tiles.append(emb_t)
            sb_ident = singles.tile([P, P], F32)
            nc.sync.dma_start(out=sb_ident, in_=ident[:, :])

            for t in range(NT):
                if t + 3 < NT:
                    issue_lstm_dma(t + 3)
                kp, b = divmod(t, BLOC)
                lstm_t = lstm_tiles.pop(t)
                # lstm work first (the DMA buffer is freed by DVE)
                prods = []
                for kk in range(2):
                    prod_t = prod_pool.tile([P, LSTM], F32, tag="prod")
                    nc.vector.tensor_mul(prod_t, lstm_t[:, kk, :], attn_l)
                    prods.append(prod_t)
                for kk in range(2):
                    col = (2 * kp + kk) * BLOC + b
                    nc.scalar.activation(
                        out=scratch,
                        in_=prods[kk],
                        func=mybir.ActivationFunctionType.Copy,
                        accum_out=tmpl[:, col : col + 1],
                    )
                for kk in range(2):
                    k = 2 * kp + kk
                    col = k * BLOC + b
                    emb_t = emb_tiles[k]
                    nc.vector.tensor_mul(emb_t[:, b, :], emb_t[:, b, :], attn_e)
                    nc.scalar.activation(
                        out=scratch[:, 0:EMB],
                        in_=emb_t[:, b, :],
                        func=mybir.ActivationFunctionType.Copy,
                        accum_out=tmpe[:, col : col + 1],
                    )

            nc.vector.tensor_add(L, tmpl, tmpe)

            # logits [8b, 512s] in PSUM via four PE transposes of [128, 8]
            # (shares the PSUM slot with scratch — scratch is done by now)
            logits = psum_pool.tile([BLOC, SEQ], F32, tag="ps")
            for k in range(NSB):
                nc.tensor.transpose(
                    out=logits[:, k * P : (k + 1) * P],
                    in_=L[:, k * BLOC : (k + 1) * BLOC],
                    identity=sb_ident,
                )

            # softmax along s (free axis)
            m = singles.tile([BLOC, 1], F32)
            nm = singles.tile([BLOC, 1], F32)
            ssum = singles.tile([BLOC, 1], F32)
            rec = singles.tile([BLOC, 1], F32)
            expt = singles.tile([BLOC, SEQ], F32)
            res = singles.tile([BLOC, SEQ], F32)
            nc.vector.reduce_max(out=m, in_=logits, axis=mybir.AxisListType.X)
            nc.vector.tensor_scalar_mul(nm, m, -1.0)
            nc.scalar.activation(
                out=expt,
                in_=logits,
                func=mybir.ActivationFunctionType.Exp,
                bias=nm,
                scale=1.0,
                accum_out=ssum,
            )
            nc.vector.reciprocal(rec, ssum)
            nc.vector.tensor_scalar_mul(res, expt, rec)
            nc.sync.dma_start(out=out[:, :], in_=res)

    nc.compile()
    return nc


_NC_CACHE = None


def _get_nc() -> bass.Bass:
    global _NC_CACHE
    if _NC_CACHE is None:
        _NC_CACHE = _build()
    return _NC_CACHE


def _make_in_maps(embedded, lstm_outputs, attn):
    embedded = np.asarray(embedded, dtype=np.float32)
    lstm_outputs = np.asarray(lstm_outputs, dtype=np.float32)
    attn = np.asarray(attn, dtype=np.float32)
    attn_bc = np.ascontiguousarray(np.broadcast_to(attn, (P, EMB + LSTM)))
    eye = np.eye(P, dtype=np.float32)
    in_maps = []
    for i in range(N_CORES):
        sl = slice(i * BLOC, (i + 1) * BLOC)
        in_maps.append(
            {
                # pre-tiled / b-major so each device tile is one
                # contiguous DRAM read
                "embedded": np.ascontiguousarray(
                    embedded[:, sl, :].reshape(NSB, P, BLOC, EMB)
                ),
                "lstm_outputs": np.ascontiguousarray(
                    lstm_outputs[:, sl, :].transpose(1, 0, 2)
                ),
                "attn_bc": attn_bc,
                "ident": eye,
            }
        )
    return in_maps


def _run(embedded, lstm_outputs, attn, trace=False, **spmd_kwargs):
    nc = _get_nc()
    in_maps = _make_in_maps(embedded, lstm_outputs, attn)
    r = run_bass_kernel_spmd(
        nc, in_maps, core_ids=list(range(N_CORES)), trace=trace, **spmd_kwargs
    )
    out = np.concatenate([r.results[i]["out"] for i in range(N_CORES)], axis=0)
    return out, r


def kernel(embedded, lstm_outputs, attn, mask=None, **_ignored) -> np.ndarray:
    out, _ = _run(embedded, lstm_outputs, attn, trace=False)
    return out.astype(np.float32)


# revision 44
# speedup vs baseline: 1.2004x; 1.2004x over previous
"""Trainium2 Bass kernel for nn_AttentionLayer_10995116278518.

Computes softmax(einsum('sbe,e->bs', embedded, attn[:300])
              + einsum('sbf,f->bs', lstm_outputs, attn[300:]), axis=1)
(the reference's mask is computed-but-discarded, so it is unused here).

Sharding: data-parallel over batch. Each of the 8 cores handles 8 of the
64 batch rows; no cross-device communication.

Per-core device kernel layout:
  rows = (s, b) pairs; tiles put 128 consecutive s on partitions for a
  fixed b, features on the free axis. A fused VectorE tensor_tensor_reduce
  (multiply by the partition-broadcast attn vector, then add-reduce along
  the free axis) produces one dot product per partition. The per-row dots
  are collected as columns of L[128s, 4k*8b]; four PE transposes turn that
  into logits [8b, 512s] in PSUM, where softmax is a pure free-axis op.
"""

import os
import sys

import numpy as np

try:
    import concourse.bass as bass
except ImportError:  # stand-alone grading dir: the runtime lives here
    sys.path.insert(0, "/opt/trn_rl_repo")
    import concourse.bass as bass

import concourse.bacc as bacc
import concourse.tile as tile
from concourse import mybir
from concourse.bass_utils import run_bass_kernel_spmd

SEQ = 512
BATCH = 64
EMB = 300
LSTM = 4096
N_CORES = 8
BLOC = BATCH // N_CORES  # 8 batch rows per core
P = 128
NSB = SEQ // P  # 4 s-blocks of 128

F32 = mybir.dt.float32


def _build() -> bass.Bass:
    nc = bacc.Bacc()
    # host passes the embedded shard pre-tiled: [NSB, 128, BLOC, EMB]
    emb = nc.declare_dram_parameter(
        "embedded", [NSB, P, BLOC, EMB], F32, isOutput=False
    )
    # lstm shard transposed to b-major: [BLOC, SEQ, LSTM]
    lstm = nc.declare_dram_parameter(
        "lstm_outputs", [BLOC, SEQ, LSTM], F32, isOutput=False
    )
    attn_bc = nc.declare_dram_parameter("attn_bc", [P, EMB + LSTM], F32, isOutput=False)
    ident = nc.declare_dram_parameter("ident", [P, P], F32, isOutput=False)
    out = nc.declare_dram_parameter("out", [BLOC, SEQ], F32, isOutput=True)

    with tile.TileContext(nc) as tc:
        with (
            tc.tile_pool(name="singles", bufs=1) as singles,
            tc.tile_pool(name="lstm_tiles", bufs=4) as lstm_pool,
            tc.tile_pool(name="emb_tiles", bufs=4) as emb_pool,
            tc.tile_pool(name="psum", bufs=1, space="PSUM") as psum_pool,
        ):
            # setup loads on the sync ring; the gpsimd ring starts on lstm
            # tiles immediately
            sb_attn = singles.tile([P, EMB + LSTM], F32)
            nc.sync.dma_start(out=sb_attn, in_=attn_bc[:, :])
            attn_e = sb_attn[:, 0:EMB]
            attn_l = sb_attn[:, EMB : EMB + LSTM]

            # per-row dot products: column k*BLOC+b holds rows (s=128k.., b)
            L = singles.tile([P, NSB * BLOC], F32)
            tmpl = singles.tile([P, NSB * BLOC], F32)  # lstm partials
            tmpe = singles.tile([P, NSB * BLOC], F32)  # embedded partials

            NT = NSB // 2 * BLOC  # 16 tiles of [128, 2, LSTM]
            lstm_tiles = {}

            def issue_lstm_dma(t):
                # both HWDGE rings (SP + ACT) — ScalarE has no compute now
                kp, b = divmod(t, BLOC)
                lt = lstm_pool.tile([P, 2, LSTM], F32, tag="lstm")
                eng = nc.sync if b % 2 == 0 else nc.scalar
                eng.dma_start(
                    out=lt,
                    in_=lstm[b, 2 * kp * P : (2 * kp + 2) * P, :].rearrange(
                        "(kk s) f -> s kk f", kk=2
                    ),
                )
                lstm_tiles[t] = lt

            # prime the pipeline; setup loads share the rings
            issue_lstm_dma(0)
            issue_lstm_dma(1)
            emb_tiles = []
            for k in range(2):
                emb_t = emb_pool.tile([P, BLOC, EMB], F32)
                nc.scalar.dma_start(out=emb_t, in_=emb[k])
                emb_tiles.append(emb_t)
            issue_lstm_dma(2)
            for k in range(2, NSB):
                emb_t = emb_pool.tile([P, BLOC, EMB], F32)
                nc.scalar.dma_start(out=emb_t, in_=emb[k])
                emb_tiles.append(emb_t)
            issue_lstm_dma(3)
            sb_ident = singles.tile([P, P], F32)
            nc.scalar.dma_start(out=sb_ident, in_=ident[:, :])

            for t in range(NT):
                if t + 4 < NT:
                    issue_lstm_dma(t + 4)
                kp, b = divmod(t, BLOC)
                lstm_t = lstm_tiles.pop(t)
                # one fused multiply+reduce per row-block on VectorE
                for kk in range(2):
                    col = (2 * kp + kk) * BLOC + b
                    nc.vector.scalar_tensor_tensor(
                        out=lstm_t[:, kk, :],
                        in0=lstm_t[:, kk, :],
                        scalar=1.0,
                        in1=attn_l,
                        op0=mybir.AluOpType.mult,
                        op1=mybir.AluOpType.mult,
                        accum_out=tmpl[:, col : col + 1],
                    )
                for kk in range(2):
                    k = 2 * kp + kk
                    col = k * BLOC + b
                    emb_t = emb_tiles[k]
                    nc.vector.scalar_tensor_tensor(
                        out=emb_t[:, b, :],
                        in0=emb_t[:, b, :],
                        scalar=1.0,
                        in1=attn_e,
                        op0=mybir.AluOpType.mult,
                        op1=mybir.AluOpType.mult,
                        accum_out=tmpe[:, col : col + 1],
                    )

            nc.vector.tensor_add(L, tmpl, tmpe)

            # logits [8b, 512s] in PSUM via four PE transposes of [128, 8]
            # (shares the PSUM slot with scratch — scratch is done by now)
            logits = psum_pool.tile([BLOC, SEQ], F32, tag="ps")
            for k in range(NSB):
                nc.tensor.transpose(
                    out=logits[:, k * P : (k + 1) * P],
                    in_=L[:, k * BLOC : (k + 1) * BLOC],
                    identity=sb_ident,
                )

            # softmax along s (free axis)
            m = singles.tile([BLOC, 1], F32)
            nm = singles.tile([BLOC, 1], F32)
            ssum = singles.tile([BLOC, 1], F32)
            rec = singles.tile([BLOC, 1], F32)
            expt = singles.tile([BLOC, SEQ], F32)
            res = singles.tile([BLOC, SEQ], F32)
            nc.vector.reduce_max(out=m, in_=logits, axis=mybir.AxisListType.X)
            nc.vector.tensor_scalar_mul(nm, m, -1.0)
            nc.scalar.activation(
                out=expt,
                in_=logits,
                func=mybir.ActivationFunctionType.Exp,
                bias=nm,
                scale=1.0,
                accum_out=ssum,
            )
            nc.vector.reciprocal(rec, ssum)
            nc.vector.tensor_scalar_mul(res, expt, rec)
            nc.sync.dma_start(out=out[:, :], in_=res)

    nc.compile()
    return nc


_NC_CACHE = None


def _get_nc() -> bass.Bass:
    global _NC_CACHE
    if _NC_CACHE is None:
        _NC_CACHE = _build()
    return _NC_CACHE


def _make_in_maps(embedded, lstm_outputs, attn):
    embedded = np.asarray(embedded, dtype=np.float32)
    lstm_outputs = np.asarray(lstm_outputs, dtype=np.float32)
    attn = np.asarray(attn, dtype=np.float32)
    attn_bc = np.ascontiguousarray(np.broadcast_to(attn, (P, EMB + LSTM)))
    eye = np.eye(P, dtype=np.float32)
    in_maps = []
    for i in range(N_CORES):
        sl = slice(i * BLOC, (i + 1) * BLOC)
        in_maps.append(
            {
                # pre-tiled / b-major so each device tile is one
                # contiguous DRAM read
                "embedded": np.ascontiguousarray(
                    embedded[:, sl, :].reshape(NSB, P, BLOC, EMB)
                ),
                "lstm_outputs": np.ascontiguousarray(
                    lstm_outputs[:, sl, :].transpose(1, 0, 2)
                ),
                "attn_bc": attn_bc,
                "ident": eye,
            }
        )
    return in_maps


def _run(embedded, lstm_outputs, attn, trace=False, **spmd_kwargs):
    nc = _get_nc()
    in_maps = _make_in_maps(embedded, lstm_outputs, attn)
    r = run_bass_kernel_spmd(
        nc, in_maps, core_ids=list(range(N_CORES)), trace=trace, **spmd_kwargs
    )
    out = np.concatenate([r.results[i]["out"] for i in range(N_CORES)], axis=0)
    return out, r


def kernel(embedded, lstm_outputs, attn, mask=None, **_ignored) -> np.ndarray:
    out, _ = _run(embedded, lstm_outputs, attn, trace=False)
    return out.astype(np.float32)


# revision 45
# speedup vs baseline: 1.2381x; 1.0314x over previous
"""Trainium2 Bass kernel for nn_AttentionLayer_10995116278518.

Computes softmax(einsum('sbe,e->bs', embedded, attn[:300])
              + einsum('sbf,f->bs', lstm_outputs, attn[300:]), axis=1)
(the reference's mask is computed-but-discarded, so it is unused here).

Sharding: data-parallel over batch. Each of the 8 cores handles 8 of the
64 batch rows; no cross-device communication.

Per-core device kernel layout:
  rows = (s, b) pairs; tiles put 128 consecutive s on partitions for a
  fixed b, features on the free axis. A fused VectorE tensor_tensor_reduce
  (multiply by the partition-broadcast attn vector, then add-reduce along
  the free axis) produces one dot product per partition. The per-row dots
  are collected as columns of L[128s, 4k*8b]; four PE transposes turn that
  into logits [8b, 512s] in PSUM, where softmax is a pure free-axis op.
"""

import os
import sys

import numpy as np

try:
    import concourse.bass as bass
except ImportError:  # stand-alone grading dir: the runtime lives here
    sys.path.insert(0, "/opt/trn_rl_repo")
    import concourse.bass as bass

import concourse.bacc as bacc
import concourse.tile as tile
from concourse import mybir
from concourse.bass_utils import run_bass_kernel_spmd

SEQ = 512
BATCH = 64
EMB = 300
LSTM = 4096
N_CORES = 8
BLOC = BATCH // N_CORES  # 8 batch rows per core
P = 128
NSB = SEQ // P  # 4 s-blocks of 128

F32 = mybir.dt.float32


def _build() -> bass.Bass:
    nc = bacc.Bacc()
    # host passes the embedded shard pre-tiled: [NSB, 128, BLOC, EMB]
    emb = nc.declare_dram_parameter(
        "embedded", [NSB, P, BLOC, EMB], F32, isOutput=False
    )
    # lstm shard transposed to b-major: [BLOC, SEQ, LSTM]
    lstm = nc.declare_dram_parameter(
        "lstm_outputs", [BLOC, SEQ, LSTM], F32, isOutput=False
    )
    attn_bc = nc.declare_dram_parameter("attn_bc", [P, EMB + LSTM], F32, isOutput=False)
    ident = nc.declare_dram_parameter("ident", [P, P], F32, isOutput=False)
    out = nc.declare_dram_parameter("out", [BLOC, SEQ], F32, isOutput=True)

    with tile.TileContext(nc) as tc:
        with (
            tc.tile_pool(name="singles", bufs=1) as singles,
            tc.tile_pool(name="lstm_tiles", bufs=7) as lstm_pool,
            tc.tile_pool(name="emb_tiles", bufs=4) as emb_pool,
            tc.tile_pool(name="psum", bufs=1, space="PSUM") as psum_pool,
        ):
            # attn loaded in two pieces: the lstm part first (it gates the
            # first fused multiply), the small embedded part separately
            sb_attn = singles.tile([P, EMB + LSTM], F32)
            attn_e = sb_attn[:, 0:EMB]
            attn_l = sb_attn[:, EMB : EMB + LSTM]
            nc.sync.dma_start(out=attn_l, in_=attn_bc[:, EMB : EMB + LSTM])
            nc.scalar.dma_start(out=attn_e, in_=attn_bc[:, 0:EMB])

            # per-row dot products: column k*BLOC+b holds rows (s=128k.., b)
            L = singles.tile([P, NSB * BLOC], F32)
            tmpl = singles.tile([P, NSB * BLOC], F32)  # lstm partials
            tmpe = singles.tile([P, NSB * BLOC], F32)  # embedded partials

            # 32 tiles of [128, LSTM]: t -> (kp, b, kk), k = 2*kp + kk
            order = []
            for kp in range(NSB // 2):
                for b in range(BLOC):
                    for kk in range(2):
                        order.append((2 * kp + kk, b))
            NT = len(order)
            lstm_tiles = {}

            def issue_lstm_dma(t):
                # both HWDGE rings (SP + ACT) — ScalarE has no compute now
                k, b = order[t]
                lt = lstm_pool.tile([P, LSTM], F32, tag="lstm")
                eng = nc.sync if b % 2 == 0 else nc.scalar
                eng.dma_start(out=lt, in_=lstm[b, k * P : (k + 1) * P, :])
                lstm_tiles[t] = lt

            # prime the pipeline; setup loads share the rings
            issue_lstm_dma(0)
            issue_lstm_dma(1)
            issue_lstm_dma(2)
            emb_tiles = []
            for k in range(2):
                emb_t = emb_pool.tile([P, BLOC, EMB], F32)
                nc.scalar.dma_start(out=emb_t, in_=emb[k])
                emb_tiles.append(emb_t)
            issue_lstm_dma(3)
            issue_lstm_dma(4)
            for k in range(2, NSB):
                emb_t = emb_pool.tile([P, BLOC, EMB], F32)
                nc.scalar.dma_start(out=emb_t, in_=emb[k])
                emb_tiles.append(emb_t)
            issue_lstm_dma(5)
            sb_ident = singles.tile([P, P], F32)
            nc.scalar.dma_start(out=sb_ident, in_=ident[:, :])

            for t in range(NT):
                if t + 6 < NT:
                    issue_lstm_dma(t + 6)
                k, b = order[t]
                col = k * BLOC + b
                lstm_t = lstm_tiles.pop(t)
                # one fused multiply+reduce per row-block on VectorE
                nc.vector.scalar_tensor_tensor(
                    out=lstm_t,
                    in0=lstm_t,
                    scalar=1.0,
                    in1=attn_l,
                    op0=mybir.AluOpType.mult,
                    op1=mybir.AluOpType.mult,
                    accum_out=tmpl[:, col : col + 1],
                )
                emb_t = emb_tiles[k]
                nc.vector.scalar_tensor_tensor(
                    out=emb_t[:, b, :],
                    in0=emb_t[:, b, :],
                    scalar=1.0,
                    in1=attn_e,
                    op0=mybir.AluOpType.mult,
                    op1=mybir.AluOpType.mult,
                    accum_out=tmpe[:, col : col + 1],
                )

            nc.vector.tensor_add(L, tmpl, tmpe)

            # logits [8b, 512s] in PSUM via four PE transposes of [128, 8]
            # (shares the PSUM slot with scratch — scratch is done by now)
            logits = psum_pool.tile([BLOC, SEQ], F32, tag="ps")
            for k in range(NSB):
                nc.tensor.transpose(
                    out=logits[:, k * P : (k + 1) * P],
                    in_=L[:, k * BLOC : (k + 1) * BLOC],
                    identity=sb_ident,
                )

            # softmax along s (free axis)
            m = singles.tile([BLOC, 1], F32)
            nm = singles.tile([BLOC, 1], F32)
            ssum = singles.tile([BLOC, 1], F32)
            rec = singles.tile([BLOC, 1], F32)
            expt = singles.tile([BLOC, SEQ], F32)
            res = singles.tile([BLOC, SEQ], F32)
            nc.vector.reduce_max(out=m, in_=logits, axis=mybir.AxisListType.X)
            nc.vector.tensor_scalar_mul(nm, m, -1.0)
            nc.scalar.activation(
                out=expt,
                in_=logits,
                func=mybir.ActivationFunctionType.Exp,
                bias=nm,
                scale=1.0,
                accum_out=ssum,
            )
            nc.vector.reciprocal(rec, ssum)
            nc.vector.tensor_scalar_mul(res, expt, rec)
            nc.sync.dma_start(out=out[:, :], in_=res)

    nc.compile()
    return nc


_NC_CACHE = None


def _get_nc() -> bass.Bass:
    global _NC_CACHE
    if _NC_CACHE is None:
        _NC_CACHE = _build()
    return _NC_CACHE


def _make_in_maps(embedded, lstm_outputs, attn):
    embedded = np.asarray(embedded, dtype=np.float32)
    lstm_outputs = np.asarray(lstm_outputs, dtype=np.float32)
    attn = np.asarray(attn, dtype=np.float32)
    attn_bc = np.ascontiguousarray(np.broadcast_to(attn, (P, EMB + LSTM)))
    eye = np.eye(P, dtype=np.float32)
    in_maps = []
    for i in range(N_CORES):
        sl = slice(i * BLOC, (i + 1) * BLOC)
        in_maps.append(
            {
                # pre-tiled / b-major so each device tile is one
                # contiguous DRAM read
                "embedded": np.ascontiguousarray(
                    embedded[:, sl, :].reshape(NSB, P, BLOC, EMB)
                ),
                "lstm_outputs": np.ascontiguousarray(
                    lstm_outputs[:, sl, :].transpose(1, 0, 2)
                ),
                "attn_bc": attn_bc,
                "ident": eye,
            }
        )
    return in_maps


def _run(embedded, lstm_outputs, attn, trace=False, **spmd_kwargs):
    nc = _get_nc()
    in_maps = _make_in_maps(embedded, lstm_outputs, attn)
    r = run_bass_kernel_spmd(
        nc, in_maps, core_ids=list(range(N_CORES)), trace=trace, **spmd_kwargs
    )
    out = np.concatenate([r.results[i]["out"] for i in range(N_CORES)], axis=0)
    return out, r


def kernel(embedded, lstm_outputs, attn, mask=None, **_ignored) -> np.ndarray:
    out, _ = _run(embedded, lstm_outputs, attn, trace=False)
    return out.astype(np.float32)
